# revision 29
# baseline (speedup 1.0000x reference)
"""Differentiable-stack kernel for Trainium2 (Bass/Tile), 8-core data parallel.

The reference soft stack only ever reads slot S-1, and the shift moves slot
s+1 -> slot s (never upward), so the output reduces to a gated linear
recurrence per (batch, d):

    y_t = a_t * y_{t-1} + b_t * x_t
    a_t = (1-p_t)(1-o_t),  b_t = p_t (1-o_t)      (scalars per (b, t))

Per core: 2 batch elements of [L=2048, D=512] f32.  The sequence is cut
into chunks of TC=127 steps; each chunk is ONE TensorE matmul with K=128:
row 0 of the moving operand is the carry y[s-1] (written there by a tiny
engine copy), rows 1..127 are x[s..s+126]:

    psum[t', d] = sum_j Ct[j, t'] * [carry; x][j, d]
    Ct[0,  t'] = prod_{k=s..s+t'} a_k           = exp(S_t')
    Ct[j', t'] = b_j * prod_{k=j+1..s+t'} a_k   = exp(S_t' - S_j + ln b_j)

(S = in-chunk cumsum of ln a; entries with j > t' are suppressed by a
-1000*max(j-t'-1,0) ramp matmul before the EXP.)  Ct tiles are built 4
chunks per PSUM group with three bf16 matmuls: S-row broadcast (hi/lo
bf16 split of S reconstructs fp32 accuracy in PSUM), bias spread via a
K=8 block-indicator, and the constant ramp.  Output rows are rotated by
one (psum row p holds t'=p-1, row 0 holds t'=126) so the next chunk's
carry is read from partition 0; the host un-rotates.  x is cast
f32->bf16 by SWDGE load DMAs; y is stored f32 by HWDGE (sync) DMAs.
"""

import os
from contextlib import ExitStack

import numpy as np

import concourse.bass as bass
import concourse.tile as tile
from concourse import bacc, mybir
from concourse.bass_utils import run_bass_kernel_spmd

F32 = mybir.dt.float32
BF16 = mybir.dt.bfloat16
ALU = mybir.AluOpType
ACTF = mybir.ActivationFunctionType

B, L, D = 16, 2048, 512
NCORES = 8
BPC = B // NCORES            # batches per core = 2
T = 128                      # matmul contraction (1 carry + 127 x rows)
TC = T - 1                   # timesteps per chunk = 127
NCH = -(-L // TC)            # chunks per batch element = 17
SEGP = 20                    # gate segments per batch (17 used, padded to 20)
SEG = BPC * SEGP             # gate-tensor partitions = 40
G4 = 4                       # chunks per Ct-build group
NG = SEGP // G4              # Ct groups per batch = 5

LGROUPS = [int(g) for g in os.environ.get("DSTACK_LG", "2,4,5,6").split(",")]
SGROUPS = [int(g) for g in os.environ.get("DSTACK_SG", "2,2,4,4,5").split(",")]
PSYC = int(os.environ.get("DSTACK_PSYC", "2"))     # chunks per psum group
PSYBUFS = int(os.environ.get("DSTACK_PSY", "3"))
CTBUFS = int(os.environ.get("DSTACK_CT", "3"))
DVE_COLS = int(os.environ.get("DSTACK_DVECOLS", "256"))  # DVE cols per 512

assert sum(LGROUPS) == NCH and sum(SGROUPS) == NCH


def build_module():
    nc = bacc.Bacc("TRN2", target_bir_lowering=False)
    xin = nc.dram_tensor("xin", [T, BPC * NCH * D], F32, kind="ExternalInput")
    pg = nc.dram_tensor("pg", [SEG, TC], F32, kind="ExternalInput")
    og = nc.dram_tensor("og", [SEG, TC], F32, kind="ExternalInput")
    yout = nc.dram_tensor("yout", [T, BPC * NCH * D], F32,
                          kind="ExternalOutput")
    # DRAM bounce for the 4-seg-grouped bias (partition reorder)
    scr_bh = nc.dram_tensor("scr_bh", [1, SEG * TC], BF16, kind="Internal")
    scr_bl = nc.dram_tensor("scr_bl", [1, SEG * TC], BF16, kind="Internal")

    with tile.TileContext(nc) as tc, ExitStack() as ctx:
        smalls = ctx.enter_context(tc.tile_pool(name="smalls", bufs=1))
        xpool = ctx.enter_context(tc.tile_pool(name="xpool", bufs=1))
        ypool = ctx.enter_context(tc.tile_pool(name="ypool", bufs=1))
        ctpool = ctx.enter_context(tc.tile_pool(name="ctpool", bufs=CTBUFS))
        pspool = ctx.enter_context(tc.tile_pool(name="pspool", bufs=1,
                                                space="PSUM"))

        # -------- gate DMAs (HWDGE sync ring, first: tiny) -----------------
        pgt = smalls.tile([SEG, TC], F32)
        ogt = smalls.tile([SEG, TC], F32)
        nc.sync.dma_start(pgt[:], pg[:])
        nc.sync.dma_start(ogt[:], og[:])

        # -------- x cast-loads (SWDGE f32->bf16); first groups early ------
        xtiles = {}          # (b, c) -> (bf16 tile, col0)
        for b in range(BPC):
            c0 = 0
            for gi, g in enumerate(LGROUPS):
                xt = xpool.tile([T, g * D], BF16, tag=f"x{b}_{gi}")
                for c in range(c0, c0 + g):
                    xtiles[(b, c)] = (xt, (c - c0) * D)
                c0 += g

        def emit_load(gi):
            g = LGROUPS[gi]
            c0 = sum(LGROUPS[:gi])
            for b in range(BPC):
                xt = xtiles[(b, c0)][0]
                lo = (b * NCH + c0) * D
                nc.gpsimd.dma_start(xt[:], xin[:, lo:lo + g * D])

        emit_load(0)
        emit_load(1)

        # -------- constants (gpsimd Q7, between load descriptor jobs) ------
        # Output-row rotation: psum row p holds t'=p-1; row 0 holds t'=126.
        # ramp: psum += sum_k L[k,j]*U4[k,(q,p)] = -1000*max(j - t'(p) - 1, 0)
        lmat = smalls.tile([T, T], BF16)
        nc.gpsimd.memset(lmat[:], 1.0)
        nc.gpsimd.affine_select(
            out=lmat[:], in_=lmat[:], compare_op=ALU.is_ge, fill=0.0,
            base=-1, pattern=[[1, T]], channel_multiplier=-1)
        umat4 = smalls.tile([T, G4, TC], BF16)
        nc.gpsimd.memset(umat4[:], -1000.0)
        nc.gpsimd.affine_select(
            out=umat4[:], in_=umat4[:], compare_op=ALU.is_ge, fill=0.0,
            base=0, pattern=[[0, G4], [-1, TC]], channel_multiplier=1)
        nc.gpsimd.affine_select(
            out=umat4[:], in_=umat4[:], compare_op=ALU.is_ge, fill=0.0,
            base=-1, pattern=[[0, G4], [1, TC]], channel_multiplier=0)
        # block indicator: blk[k, (q, p)] = 1{k == q mod G4}  (bias spread)
        blk = smalls.tile([2 * G4, G4, TC], BF16)
        nc.gpsimd.memset(blk[:], 0.0)
        nc.gpsimd.affine_select(
            out=blk[:], in_=blk[:], compare_op=ALU.not_equal, fill=1.0,
            base=0, pattern=[[-1, G4], [0, TC]], channel_multiplier=1)
        nc.gpsimd.affine_select(
            out=blk[:], in_=blk[:], compare_op=ALU.not_equal, fill=1.0,
            base=-G4, pattern=[[-1, G4], [0, TC]], channel_multiplier=1)

        emit_load(2)
        emit_load(3)

        # -------- gate math (tiny, [SEG, TC]) ------------------------------
        ones_st = smalls.tile([SEG, TC], F32)
        nc.vector.memset(ones_st[:], 1.0)
        ones_row = smalls.tile([2, T], BF16)
        nc.vector.memset(ones_row[:], 1.0)

        om = smalls.tile([SEG, TC], F32)
        av = smalls.tile([SEG, TC], F32)
        bv = smalls.tile([SEG, TC], F32)
        nc.vector.tensor_scalar(om[:], ogt[:], -1.0, 1.0, ALU.mult, ALU.add)
        nc.vector.tensor_scalar(av[:], pgt[:], -1.0, 1.0, ALU.mult, ALU.add)
        nc.vector.tensor_mul(av[:], av[:], om[:])
        nc.vector.tensor_mul(bv[:], pgt[:], om[:])
        nc.vector.tensor_scalar(av[:], av[:], 1e-30, None, ALU.max)
        nc.vector.tensor_scalar(bv[:], bv[:], 1e-30, None, ALU.max)

        la = smalls.tile([SEG, TC], F32)
        nc.scalar.activation(la[:], av[:], ACTF.Ln)
        lb = smalls.tile([SEG, TC], F32)
        nc.scalar.activation(lb[:], bv[:], ACTF.Ln)

        sv = smalls.tile([SEG, TC], F32)
        nc.vector.tensor_tensor_scan(sv[:], ones_st[:], la[:], 0.0,
                                     ALU.mult, ALU.add)
        bias = smalls.tile([SEG, TC], F32)
        nc.vector.tensor_sub(bias[:], lb[:], sv[:])

        # hi/lo bf16 split (fp32 matmuls are dual-pass; two bf16 matmuls
        # reconstruct fp32-accurate sums in the f32 PSUM)
        svh = smalls.tile([SEG, TC], BF16)
        svl = smalls.tile([SEG, TC], BF16)
        nc.vector.tensor_copy(svh[:], sv[:])
        nc.vector.tensor_sub(svl[:], sv[:], svh[:])
        bih = smalls.tile([SEG, TC], BF16)
        bil = smalls.tile([SEG, TC], BF16)
        nc.vector.tensor_copy(bih[:], bias[:])
        nc.vector.tensor_sub(bil[:], bias[:], bih[:])

        # rotate t' by one in SBUF (free-dim copies), then partition-0 row
        # layout via direct SBUF->SBUF DMAs (scalar HWDGE ring)
        svhr = smalls.tile([SEG, TC], BF16)
        svlr = smalls.tile([SEG, TC], BF16)
        for dst, src in ((svhr, svh), (svlr, svl)):
            nc.vector.tensor_copy(dst[:, 0:1], src[:, TC - 1:TC])
            nc.vector.tensor_copy(dst[:, 1:TC], src[:, 0:TC - 1])
        srows2 = smalls.tile([2, SEG * TC], BF16)
        r2 = srows2[:].rearrange("a (p f) -> a p f", f=TC)
        nc.scalar.dma_start(r2[0:1], svhr[:])
        nc.scalar.dma_start(r2[1:2], svlr[:])

        # grouped bias via DRAM bounce; j'=0 (carry row) bias is 0
        bghl = smalls.tile([2 * G4, 2 * NG, T], BF16)
        nc.vector.memset(bghl[:, :, 0:1], 0.0)
        nc.scalar.dma_start(scr_bh[:].rearrange("o (p f) -> (o p) f", f=TC),
                            bih[:])
        nc.scalar.dma_start(scr_bl[:].rearrange("o (p f) -> (o p) f", f=TC),
                            bil[:])
        nc.scalar.dma_start(
            bghl[0:G4, :, 1:T],
            scr_bh[:].rearrange("o (g p f) -> (o p) g f", p=G4, f=TC))
        nc.scalar.dma_start(
            bghl[G4:2 * G4, :, 1:T],
            scr_bl[:].rearrange("o (g p f) -> (o p) g f", p=G4, f=TC))

        # -------- store plan -----------------------------------------------
        sgrp = {}
        for b in range(BPC):
            c0 = 0
            for gi, g in enumerate(SGROUPS):
                yt = ypool.tile([T, g * D], F32, tag=f"y{b}_{gi}")
                for c in range(c0, c0 + g):
                    sgrp[(b, c)] = (yt, (c - c0) * D, c == c0 + g - 1,
                                    (b * NCH + c0) * D, g)
                c0 += g

        # -------- main loop ------------------------------------------------
        cts = {}
        psys = {}
        for c in range(NCH):
            for b in range(BPC):
                seg = b * SEGP + c
                q = c // G4
                if c % G4 == 0:
                    gsz = min(G4, NCH - c)
                    w = gsz * TC
                    ps2 = pspool.tile([T, G4 * TC], F32, tag="p2", bufs=2,
                                      name=f"ps2_{b}_{c}")
                    nc.tensor.matmul(ps2[:, 0:w], ones_row[:, 0:T],
                                     srows2[:, seg * TC:(seg + gsz) * TC],
                                     start=True, stop=False)
                    nc.tensor.matmul(
                        ps2[:, 0:w], bghl[:, b * NG + q, :],
                        blk[:].rearrange("p a b -> p (a b)")[:, 0:w],
                        start=False, stop=False, skip_group_check=True)
                    nc.tensor.matmul(
                        ps2[:, 0:w], lmat[:],
                        umat4[:].rearrange("p a b -> p (a b)")[:, 0:w],
                        start=False, stop=True, skip_group_check=True)
                    ct = ctpool.tile([T, G4 * TC], BF16, tag=f"ct{b}",
                                     name=f"ct_{b}_{c}")
                    nc.scalar.activation(ct[:, 0:w], ps2[:, 0:w], ACTF.Exp)
                    cts[(b, q)] = ct

                ct = cts[(b, q)]
                xt, xcol = xtiles[(b, c)]
                if c % PSYC == 0:
                    psy = pspool.tile([T, PSYC * D], F32, tag="psy",
                                      bufs=PSYBUFS, name=f"psy{b}_{c}")
                    psys[b] = psy
                psy = psys[b]
                pcol = (c % PSYC) * D
                nc.tensor.matmul(psy[0:TC, pcol:pcol + D],
                                 ct[:, (c % G4) * TC:(c % G4 + 1) * TC],
                                 xt[:, xcol:xcol + D],
                                 start=True, stop=True)

                # carry for the next chunk: rotated row 0 holds t'=126; its
                # own carry coefficient is exp(S_126) ~ e^-264 = 0, so this
                # never waits on the previous carry.  Write it straight into
                # row 0 (the carry slot) of the next chunk's x tile.
                if c < NCH - 1:
                    xt1, xcol1 = xtiles[(b, c + 1)]
                    if c % 2 == 0:
                        nc.scalar.copy(xt1[0:1, xcol1:xcol1 + D],
                                       psy[0:1, pcol:pcol + D])
                    else:
                        nc.vector.tensor_copy(xt1[0:1, xcol1:xcol1 + D],
                                              psy[0:1, pcol:pcol + D])

                # psum group -> sbuf y (split ScalarE / DVE), once per group
                if c % PSYC == PSYC - 1 or c == NCH - 1:
                    cg0 = c - (c % PSYC)
                    yt, ycol, _, _, _ = sgrp[(b, cg0)]
                    w = (c % PSYC + 1) * D
                    nds = DVE_COLS * w // 512
                    nc.scalar.copy(yt[:, ycol:ycol + w - nds],
                                   psy[:, 0:w - nds])
                    if nds:
                        nc.vector.tensor_copy(yt[:, ycol + w - nds:ycol + w],
                                              psy[:, w - nds:w])
                yt, ycol, last, dcol0, g = sgrp[(b, c)]
                if last:
                    # stored rotated (row p holds t'=p-1); host un-rotates
                    nc.sync.dma_start(yout[:, dcol0:dcol0 + g * D], yt[:])

    nc.compile()
    return nc


_module_cache = {}


def _get_module():
    if "nc" not in _module_cache:
        _module_cache["nc"] = build_module()
    return _module_cache["nc"]


def make_in_maps(x, push_gate, pop_gate):
    x = np.ascontiguousarray(np.asarray(x), dtype=np.float32)
    pgf = np.asarray(push_gate, dtype=np.float32).reshape(B, L)
    ogf = np.asarray(pop_gate, dtype=np.float32).reshape(B, L)
    LP = NCH * TC                      # padded length = 2159
    xp = np.zeros((B, LP + 1, D), dtype=np.float32)
    xp[:, 1:L + 1] = x                 # shift by one: row 0 = carry slot
    gp = np.zeros((B, LP), dtype=np.float32)
    op_ = np.zeros((B, LP), dtype=np.float32)
    gp[:, :L] = pgf
    op_[:, :L] = ogf
    in_maps = []
    for i in range(NCORES):
        sl = slice(i * BPC, (i + 1) * BPC)
        # xin[p, b, c, d] = xp[b, TC*c + p]  (p=0 -> carry slot)
        xi = np.zeros((T, BPC, NCH, D), dtype=np.float32)
        xpc = xp[sl]
        for c in range(NCH):
            xi[:, :, c, :] = xpc[:, TC * c:TC * c + T].transpose(1, 0, 2)
        gg = np.zeros((SEG, TC), dtype=np.float32)
        oo = np.zeros((SEG, TC), dtype=np.float32)
        gg.reshape(BPC, SEGP, TC)[:, :NCH] = gp[sl].reshape(BPC, NCH, TC)
        oo.reshape(BPC, SEGP, TC)[:, :NCH] = op_[sl].reshape(BPC, NCH, TC)
        in_maps.append({
            "xin": np.ascontiguousarray(xi.reshape(T, BPC * NCH * D)),
            "pg": np.ascontiguousarray(gg),
            "og": np.ascontiguousarray(oo),
        })
    return in_maps


def run(x, push_gate, pop_gate, **spmd_kwargs):
    """Run on hardware; returns (output, BassKernelResults)."""
    nc = _get_module()
    in_maps = make_in_maps(x, push_gate, pop_gate)
    res = run_bass_kernel_spmd(nc, in_maps, core_ids=list(range(NCORES)),
                               **spmd_kwargs)
    outs = []
    for i in range(NCORES):
        yo = res.results[i]["yout"].reshape(T, BPC, NCH, D)
        # row p holds t'=p-1 (row 0 holds t'=TC-1); un-rotate and trim
        yr = np.concatenate([yo[1:TC], yo[0:1]], axis=0)
        y = yr.transpose(1, 2, 0, 3).reshape(BPC, NCH * TC, D)[:, :L]
        outs.append(y)
    return np.concatenate(outs, axis=0), res


def kernel(x, push_gate, pop_gate):
    out, _ = run(x, push_gate, pop_gate)
    return out


# revision 30
# speedup vs baseline: 1.0649x; 1.0649x over previous
"""Differentiable-stack kernel for Trainium2 (Bass/Tile), 8-core data parallel.

The reference soft stack only ever reads slot S-1, and the shift moves slot
s+1 -> slot s (never upward), so the output reduces to a gated linear
recurrence per (batch, d):

    y_t = a_t * y_{t-1} + b_t * x_t
    a_t = (1-p_t)(1-o_t),  b_t = p_t (1-o_t)      (scalars per (b, t))

Per core: 2 batch elements of [L=2048, D=512] f32.  The sequence is cut
into chunks of TC=127 steps; each chunk is ONE TensorE matmul with K=128:
row 0 of the moving operand is the carry y[s-1] (written there by a tiny
engine copy), rows 1..127 are x[s..s+126]:

    psum[t', d] = sum_j Ct[j, t'] * [carry; x][j, d]
    Ct[0,  t'] = prod_{k=s..s+t'} a_k           = exp(S_t')
    Ct[j', t'] = b_j * prod_{k=j+1..s+t'} a_k   = exp(S_t' - S_j + ln b_j)

(S = in-chunk cumsum of ln a; entries with j > t' are suppressed by a
-1000*max(j-t'-1,0) ramp matmul before the EXP.)  Ct tiles are built 4
chunks per PSUM group with three bf16 matmuls: S-row broadcast (hi/lo
bf16 split of S reconstructs fp32 accuracy in PSUM), bias spread via a
K=8 block-indicator, and the constant ramp.  Output rows are rotated by
one (psum row p holds t'=p-1, row 0 holds t'=126) so the next chunk's
carry is read from partition 0; the host un-rotates.  x is cast
f32->bf16 by SWDGE load DMAs; y is stored f32 by HWDGE (sync) DMAs.
"""

import os
from contextlib import ExitStack

import numpy as np

import concourse.bass as bass
import concourse.tile as tile
from concourse import bacc, mybir
from concourse.bass_utils import run_bass_kernel_spmd

F32 = mybir.dt.float32
BF16 = mybir.dt.bfloat16
ALU = mybir.AluOpType
ACTF = mybir.ActivationFunctionType

B, L, D = 16, 2048, 512
NCORES = 8
BPC = B // NCORES            # batches per core = 2
T = 128                      # matmul contraction (1 carry + 127 x rows)
TC = T - 1                   # timesteps per chunk = 127
NCH = -(-L // TC)            # chunks per batch element = 17
SEGP = 20                    # gate segments per batch (17 used, padded to 20)
SEG = BPC * SEGP             # gate-tensor partitions = 40
G4 = 4                       # chunks per Ct-build group
NG = SEGP // G4              # Ct groups per batch = 5

LGROUPS = [int(g) for g in os.environ.get("DSTACK_LG", "2,4,5,6").split(",")]
SGROUPS = [int(g) for g in os.environ.get("DSTACK_SG", "2,2,4,4,5").split(",")]
PSYC = int(os.environ.get("DSTACK_PSYC", "2"))     # chunks per psum group
PSYBUFS = int(os.environ.get("DSTACK_PSY", "3"))
CTBUFS = int(os.environ.get("DSTACK_CT", "3"))
DVE_COLS = int(os.environ.get("DSTACK_DVECOLS", "256"))  # DVE cols per 512

assert sum(LGROUPS) == NCH and sum(SGROUPS) == NCH


def build_module():
    nc = bacc.Bacc("TRN2", target_bir_lowering=False)
    xin = nc.dram_tensor("xin", [T, BPC * NCH * D], F32, kind="ExternalInput")
    pg = nc.dram_tensor("pg", [SEG, TC], F32, kind="ExternalInput")
    og = nc.dram_tensor("og", [SEG, TC], F32, kind="ExternalInput")
    yout = nc.dram_tensor("yout", [T, BPC * NCH * D], F32,
                          kind="ExternalOutput")
    # DRAM bounce for the 4-seg-grouped bias (partition reorder)
    scr_bh = nc.dram_tensor("scr_bh", [1, SEG * TC], BF16, kind="Internal")
    scr_bl = nc.dram_tensor("scr_bl", [1, SEG * TC], BF16, kind="Internal")

    with tile.TileContext(nc) as tc, ExitStack() as ctx:
        smalls = ctx.enter_context(tc.tile_pool(name="smalls", bufs=1))
        xpool = ctx.enter_context(tc.tile_pool(name="xpool", bufs=1))
        ypool = ctx.enter_context(tc.tile_pool(name="ypool", bufs=1))
        ctpool = ctx.enter_context(tc.tile_pool(name="ctpool", bufs=CTBUFS))
        pspool = ctx.enter_context(tc.tile_pool(name="pspool", bufs=1,
                                                space="PSUM"))

        # -------- gate DMAs (HWDGE sync ring, first: tiny) -----------------
        pgt = smalls.tile([SEG, TC], F32)
        ogt = smalls.tile([SEG, TC], F32)
        nc.sync.dma_start(pgt[:], pg[:])
        nc.sync.dma_start(ogt[:], og[:])

        # -------- x loads (HWDGE f32, sync ring); engines cast to bf16 ----
        xtiles = {}          # (b, c) -> (bf16 tile, col0)
        xf32 = {}
        for b in range(BPC):
            c0 = 0
            for gi, g in enumerate(LGROUPS):
                xt = xpool.tile([T, g * D], BF16, tag=f"x{b}_{gi}")
                xf = xpool.tile([T, g * D], F32, tag=f"xf{b}_{gi}")
                xf32[(b, gi)] = xf
                for c in range(c0, c0 + g):
                    xtiles[(b, c)] = (xt, (c - c0) * D)
                c0 += g
        for gi, g in enumerate(LGROUPS):
            c0 = sum(LGROUPS[:gi])
            for b in range(BPC):
                lo = (b * NCH + c0) * D
                nc.sync.dma_start(xf32[(b, gi)][:], xin[:, lo:lo + g * D])

        # -------- constants (gpsimd Q7, between load descriptor jobs) ------
        # Output-row rotation: psum row p holds t'=p-1; row 0 holds t'=126.
        # ramp: psum += sum_k L[k,j]*U4[k,(q,p)] = -1000*max(j - t'(p) - 1, 0)
        lmat = smalls.tile([T, T], BF16)
        nc.gpsimd.memset(lmat[:], 1.0)
        nc.gpsimd.affine_select(
            out=lmat[:], in_=lmat[:], compare_op=ALU.is_ge, fill=0.0,
            base=-1, pattern=[[1, T]], channel_multiplier=-1)
        umat4 = smalls.tile([T, G4, TC], BF16)
        nc.gpsimd.memset(umat4[:], -1000.0)
        nc.gpsimd.affine_select(
            out=umat4[:], in_=umat4[:], compare_op=ALU.is_ge, fill=0.0,
            base=0, pattern=[[0, G4], [-1, TC]], channel_multiplier=1)
        nc.gpsimd.affine_select(
            out=umat4[:], in_=umat4[:], compare_op=ALU.is_ge, fill=0.0,
            base=-1, pattern=[[0, G4], [1, TC]], channel_multiplier=0)
        # block indicator: blk[k, (q, p)] = 1{k == q mod G4}  (bias spread)
        blk = smalls.tile([2 * G4, G4, TC], BF16)
        nc.gpsimd.memset(blk[:], 0.0)
        nc.gpsimd.affine_select(
            out=blk[:], in_=blk[:], compare_op=ALU.not_equal, fill=1.0,
            base=0, pattern=[[-1, G4], [0, TC]], channel_multiplier=1)
        nc.gpsimd.affine_select(
            out=blk[:], in_=blk[:], compare_op=ALU.not_equal, fill=1.0,
            base=-G4, pattern=[[-1, G4], [0, TC]], channel_multiplier=1)

        # -------- gate math (tiny, [SEG, TC]) ------------------------------
        ones_st = smalls.tile([SEG, TC], F32)
        nc.vector.memset(ones_st[:], 1.0)
        ones_row = smalls.tile([2, T], BF16)
        nc.vector.memset(ones_row[:], 1.0)

        om = smalls.tile([SEG, TC], F32)
        av = smalls.tile([SEG, TC], F32)
        bv = smalls.tile([SEG, TC], F32)
        nc.vector.tensor_scalar(om[:], ogt[:], -1.0, 1.0, ALU.mult, ALU.add)
        nc.vector.tensor_scalar(av[:], pgt[:], -1.0, 1.0, ALU.mult, ALU.add)
        nc.vector.tensor_mul(av[:], av[:], om[:])
        nc.vector.tensor_mul(bv[:], pgt[:], om[:])
        nc.vector.tensor_scalar(av[:], av[:], 1e-30, None, ALU.max)
        nc.vector.tensor_scalar(bv[:], bv[:], 1e-30, None, ALU.max)

        la = smalls.tile([SEG, TC], F32)
        nc.scalar.activation(la[:], av[:], ACTF.Ln)
        lb = smalls.tile([SEG, TC], F32)
        nc.scalar.activation(lb[:], bv[:], ACTF.Ln)

        sv = smalls.tile([SEG, TC], F32)
        nc.vector.tensor_tensor_scan(sv[:], ones_st[:], la[:], 0.0,
                                     ALU.mult, ALU.add)
        bias = smalls.tile([SEG, TC], F32)
        nc.vector.tensor_sub(bias[:], lb[:], sv[:])

        # hi/lo bf16 split (fp32 matmuls are dual-pass; two bf16 matmuls
        # reconstruct fp32-accurate sums in the f32 PSUM)
        svh = smalls.tile([SEG, TC], BF16)
        svl = smalls.tile([SEG, TC], BF16)
        nc.vector.tensor_copy(svh[:], sv[:])
        nc.vector.tensor_sub(svl[:], sv[:], svh[:])
        bih = smalls.tile([SEG, TC], BF16)
        bil = smalls.tile([SEG, TC], BF16)
        nc.vector.tensor_copy(bih[:], bias[:])
        nc.vector.tensor_sub(bil[:], bias[:], bih[:])

        # rotate t' by one in SBUF (free-dim copies), then partition-0 row
        # layout via direct SBUF->SBUF DMAs (scalar HWDGE ring)
        svhr = smalls.tile([SEG, TC], BF16)
        svlr = smalls.tile([SEG, TC], BF16)
        for dst, src in ((svhr, svh), (svlr, svl)):
            nc.vector.tensor_copy(dst[:, 0:1], src[:, TC - 1:TC])
            nc.vector.tensor_copy(dst[:, 1:TC], src[:, 0:TC - 1])
        srows2 = smalls.tile([2, SEG * TC], BF16)
        r2 = srows2[:].rearrange("a (p f) -> a p f", f=TC)
        nc.scalar.dma_start(r2[0:1], svhr[:])
        nc.scalar.dma_start(r2[1:2], svlr[:])

        # grouped bias via DRAM bounce; j'=0 (carry row) bias is 0
        bghl = smalls.tile([2 * G4, 2 * NG, T], BF16)
        nc.vector.memset(bghl[:, :, 0:1], 0.0)
        nc.scalar.dma_start(scr_bh[:].rearrange("o (p f) -> (o p) f", f=TC),
                            bih[:])
        nc.scalar.dma_start(scr_bl[:].rearrange("o (p f) -> (o p) f", f=TC),
                            bil[:])
        nc.scalar.dma_start(
            bghl[0:G4, :, 1:T],
            scr_bh[:].rearrange("o (g p f) -> (o p) g f", p=G4, f=TC))
        nc.scalar.dma_start(
            bghl[G4:2 * G4, :, 1:T],
            scr_bl[:].rearrange("o (g p f) -> (o p) g f", p=G4, f=TC))

        # -------- PE warmup: back-to-back dummy matmuls during load window
        nwarm = int(os.environ.get("DSTACK_WARM", "20"))
        for wi in range(nwarm):
            pw = pspool.tile([T, G4 * TC], F32, tag="p2", bufs=2,
                             name=f"warm{wi}")
            nc.tensor.matmul(pw[:, 0:G4 * TC], lmat[:],
                             umat4[:].rearrange("p a b -> p (a b)"),
                             start=True, stop=True)

        # -------- store plan -----------------------------------------------
        sgrp = {}
        for b in range(BPC):
            c0 = 0
            for gi, g in enumerate(SGROUPS):
                yt = ypool.tile([T, g * D], F32, tag=f"y{b}_{gi}")
                for c in range(c0, c0 + g):
                    sgrp[(b, c)] = (yt, (c - c0) * D, c == c0 + g - 1,
                                    (b * NCH + c0) * D, g)
                c0 += g

        # -------- main loop ------------------------------------------------
        cts = {}
        psys = {}
        gstart = {}
        for gi in range(len(LGROUPS)):
            st = sum(LGROUPS[:gi])
            gstart[max(0, st - 1)] = gi
        for c in range(NCH):
            for b in range(BPC):
                seg = b * SEGP + c
                q = c // G4
                if c in gstart:
                    gi = gstart[c]
                    g = LGROUPS[gi]
                    xf = xf32[(b, gi)]
                    xtt = xtiles[(b, sum(LGROUPS[:gi]))][0]
                    h = g * D * 3 // 4
                    nc.vector.tensor_copy(xtt[:, 0:h], xf[:, 0:h])
                    nc.gpsimd.tensor_copy(xtt[:, h:g * D], xf[:, h:g * D])
                if c % G4 == 0:
                    gsz = min(G4, NCH - c)
                    w = gsz * TC
                    ps2 = pspool.tile([T, G4 * TC], F32, tag="p2", bufs=2,
                                      name=f"ps2_{b}_{c}")
                    nc.tensor.matmul(ps2[:, 0:w], ones_row[:, 0:T],
                                     srows2[:, seg * TC:(seg + gsz) * TC],
                                     start=True, stop=False)
                    nc.tensor.matmul(
                        ps2[:, 0:w], bghl[:, b * NG + q, :],
                        blk[:].rearrange("p a b -> p (a b)")[:, 0:w],
                        start=False, stop=False, skip_group_check=True)
                    nc.tensor.matmul(
                        ps2[:, 0:w], lmat[:],
                        umat4[:].rearrange("p a b -> p (a b)")[:, 0:w],
                        start=False, stop=True, skip_group_check=True)
                    ct = ctpool.tile([T, G4 * TC], BF16, tag=f"ct{b}",
                                     name=f"ct_{b}_{c}")
                    nc.scalar.activation(ct[:, 0:w], ps2[:, 0:w], ACTF.Exp)
                    cts[(b, q)] = ct

                ct = cts[(b, q)]
                xt, xcol = xtiles[(b, c)]
                if c % PSYC == 0:
                    psy = pspool.tile([T, PSYC * D], F32, tag="psy",
                                      bufs=PSYBUFS, name=f"psy{b}_{c}")
                    psys[b] = psy
                psy = psys[b]
                pcol = (c % PSYC) * D
                nc.tensor.matmul(psy[0:TC, pcol:pcol + D],
                                 ct[:, (c % G4) * TC:(c % G4 + 1) * TC],
                                 xt[:, xcol:xcol + D],
                                 start=True, stop=True)

                # carry for the next chunk: rotated row 0 holds t'=126; its
                # own carry coefficient is exp(S_126) ~ e^-264 = 0, so this
                # never waits on the previous carry.  Write it straight into
                # row 0 (the carry slot) of the next chunk's x tile.
                if c < NCH - 1:
                    xt1, xcol1 = xtiles[(b, c + 1)]
                    if c % 2 == 0:
                        nc.scalar.copy(xt1[0:1, xcol1:xcol1 + D],
                                       psy[0:1, pcol:pcol + D])
                    else:
                        nc.vector.tensor_copy(xt1[0:1, xcol1:xcol1 + D],
                                              psy[0:1, pcol:pcol + D])

                # psum group -> sbuf y (split ScalarE / DVE), once per group
                if c % PSYC == PSYC - 1 or c == NCH - 1:
                    cg0 = c - (c % PSYC)
                    yt, ycol, _, _, _ = sgrp[(b, cg0)]
                    w = (c % PSYC + 1) * D
                    nds = DVE_COLS * w // 512
                    nc.scalar.copy(yt[:, ycol:ycol + w - nds],
                                   psy[:, 0:w - nds])
                    if nds:
                        nc.vector.tensor_copy(yt[:, ycol + w - nds:ycol + w],
                                              psy[:, w - nds:w])
                yt, ycol, last, dcol0, g = sgrp[(b, c)]
                if last:
                    # stored rotated (row p holds t'=p-1); host un-rotates
                    nc.sync.dma_start(yout[:, dcol0:dcol0 + g * D], yt[:])

    nc.compile()
    return nc


_module_cache = {}


def _get_module():
    if "nc" not in _module_cache:
        _module_cache["nc"] = build_module()
    return _module_cache["nc"]


def make_in_maps(x, push_gate, pop_gate):
    x = np.ascontiguousarray(np.asarray(x), dtype=np.float32)
    pgf = np.asarray(push_gate, dtype=np.float32).reshape(B, L)
    ogf = np.asarray(pop_gate, dtype=np.float32).reshape(B, L)
    LP = NCH * TC                      # padded length = 2159
    xp = np.zeros((B, LP + 1, D), dtype=np.float32)
    xp[:, 1:L + 1] = x                 # shift by one: row 0 = carry slot
    gp = np.zeros((B, LP), dtype=np.float32)
    op_ = np.zeros((B, LP), dtype=np.float32)
    gp[:, :L] = pgf
    op_[:, :L] = ogf
    in_maps = []
    for i in range(NCORES):
        sl = slice(i * BPC, (i + 1) * BPC)
        # xin[p, b, c, d] = xp[b, TC*c + p]  (p=0 -> carry slot)
        xi = np.zeros((T, BPC, NCH, D), dtype=np.float32)
        xpc = xp[sl]
        for c in range(NCH):
            xi[:, :, c, :] = xpc[:, TC * c:TC * c + T].transpose(1, 0, 2)
        gg = np.zeros((SEG, TC), dtype=np.float32)
        oo = np.zeros((SEG, TC), dtype=np.float32)
        gg.reshape(BPC, SEGP, TC)[:, :NCH] = gp[sl].reshape(BPC, NCH, TC)
        oo.reshape(BPC, SEGP, TC)[:, :NCH] = op_[sl].reshape(BPC, NCH, TC)
        in_maps.append({
            "xin": np.ascontiguousarray(xi.reshape(T, BPC * NCH * D)),
            "pg": np.ascontiguousarray(gg),
            "og": np.ascontiguousarray(oo),
        })
    return in_maps


def run(x, push_gate, pop_gate, **spmd_kwargs):
    """Run on hardware; returns (output, BassKernelResults)."""
    nc = _get_module()
    in_maps = make_in_maps(x, push_gate, pop_gate)
    res = run_bass_kernel_spmd(nc, in_maps, core_ids=list(range(NCORES)),
                               **spmd_kwargs)
    outs = []
    for i in range(NCORES):
        yo = res.results[i]["yout"].reshape(T, BPC, NCH, D)
        # row p holds t'=p-1 (row 0 holds t'=TC-1); un-rotate and trim
        yr = np.concatenate([yo[1:TC], yo[0:1]], axis=0)
        y = yr.transpose(1, 2, 0, 3).reshape(BPC, NCH * TC, D)[:, :L]
        outs.append(y)
    return np.concatenate(outs, axis=0), res


def kernel(x, push_gate, pop_gate):
    out, _ = run(x, push_gate, pop_gate)
    return out


# revision 33
# speedup vs baseline: 1.0665x; 1.0015x over previous
"""Differentiable-stack kernel for Trainium2 (Bass/Tile), 8-core data parallel.

The reference soft stack only ever reads slot S-1, and the shift moves slot
s+1 -> slot s (never upward), so the output reduces to a gated linear
recurrence per (batch, d):

    y_t = a_t * y_{t-1} + b_t * x_t
    a_t = (1-p_t)(1-o_t),  b_t = p_t (1-o_t)      (scalars per (b, t))

Per core: 2 batch elements of [L=2048, D=512] f32.  The sequence is cut
into chunks of TC=127 steps; each chunk is ONE TensorE matmul with K=128:
row 0 of the moving operand is the carry y[s-1] (written there by a tiny
engine copy), rows 1..127 are x[s..s+126]:

    psum[t', d] = sum_j Ct[j, t'] * [carry; x][j, d]
    Ct[0,  t'] = prod_{k=s..s+t'} a_k           = exp(S_t')
    Ct[j', t'] = b_j * prod_{k=j+1..s+t'} a_k   = exp(S_t' - S_j + ln b_j)

(S = in-chunk cumsum of ln a; entries with j > t' are suppressed by a
-1000*max(j-t'-1,0) ramp matmul before the EXP.)  Ct tiles are built 4
chunks per PSUM group with three bf16 matmuls: S-row broadcast (hi/lo
bf16 split of S reconstructs fp32 accuracy in PSUM), bias spread via a
K=8 block-indicator, and the constant ramp.  Output rows are rotated by
one (psum row p holds t'=p-1, row 0 holds t'=126) so the next chunk's
carry is read from partition 0; the host un-rotates.  x is cast
f32->bf16 by SWDGE load DMAs; y is stored f32 by HWDGE (sync) DMAs.
"""

import os
from contextlib import ExitStack

import numpy as np

import concourse.bass as bass
import concourse.tile as tile
from concourse import bacc, mybir
from concourse.bass_utils import run_bass_kernel_spmd

F32 = mybir.dt.float32
BF16 = mybir.dt.bfloat16
ALU = mybir.AluOpType
ACTF = mybir.ActivationFunctionType

B, L, D = 16, 2048, 512
NCORES = 8
BPC = B // NCORES            # batches per core = 2
T = 128                      # matmul contraction (1 carry + 127 x rows)
TC = T - 1                   # timesteps per chunk = 127
NCH = -(-L // TC)            # chunks per batch element = 17
SEGP = 20                    # gate segments per batch (17 used, padded to 20)
SEG = BPC * SEGP             # gate-tensor partitions = 40
G4 = 4                       # chunks per Ct-build group
NG = SEGP // G4              # Ct groups per batch = 5

LGROUPS = [int(g) for g in os.environ.get("DSTACK_LG", "1,1,2,2,2,2,2,2,3").split(",")]
SGROUPS = [int(g) for g in os.environ.get("DSTACK_SG", "2,2,4,4,4,1").split(",")]
PSYC = int(os.environ.get("DSTACK_PSYC", "2"))     # chunks per psum group
PSYBUFS = int(os.environ.get("DSTACK_PSY", "3"))
CTBUFS = int(os.environ.get("DSTACK_CT", "3"))
DVE_COLS = int(os.environ.get("DSTACK_DVECOLS", "192"))  # DVE cols per 512

assert sum(LGROUPS) == NCH and sum(SGROUPS) == NCH


def build_module():
    nc = bacc.Bacc("TRN2", target_bir_lowering=False)
    xin = nc.dram_tensor("xin", [T, BPC * NCH * D], F32, kind="ExternalInput")
    pg = nc.dram_tensor("pg", [SEG, TC], F32, kind="ExternalInput")
    og = nc.dram_tensor("og", [SEG, TC], F32, kind="ExternalInput")
    yout = nc.dram_tensor("yout", [T, BPC * NCH * D], F32,
                          kind="ExternalOutput")
    # DRAM bounce for the 4-seg-grouped bias (partition reorder)
    scr_bh = nc.dram_tensor("scr_bh", [1, SEG * TC], BF16, kind="Internal")
    scr_bl = nc.dram_tensor("scr_bl", [1, SEG * TC], BF16, kind="Internal")

    with tile.TileContext(nc) as tc, ExitStack() as ctx:
        smalls = ctx.enter_context(tc.tile_pool(name="smalls", bufs=1))
        xpool = ctx.enter_context(tc.tile_pool(name="xpool", bufs=1))
        ypool = ctx.enter_context(tc.tile_pool(name="ypool", bufs=1))
        ctpool = ctx.enter_context(tc.tile_pool(name="ctpool", bufs=CTBUFS))
        pspool = ctx.enter_context(tc.tile_pool(name="pspool", bufs=1,
                                                space="PSUM"))

        # -------- gate DMAs (HWDGE sync ring, first: tiny) -----------------
        pgt = smalls.tile([SEG, TC], F32)
        ogt = smalls.tile([SEG, TC], F32)
        nc.sync.dma_start(pgt[:], pg[:])
        nc.sync.dma_start(ogt[:], og[:])

        # -------- x loads (HWDGE f32, sync ring); engines cast to bf16 ----
        xtiles = {}          # (b, c) -> (bf16 tile, col0)
        xf32 = {}
        for b in range(BPC):
            c0 = 0
            for gi, g in enumerate(LGROUPS):
                xt = xpool.tile([T, g * D], BF16, tag=f"x{b}_{gi}")
                xf = xpool.tile([T, g * D], F32, tag=f"xf{b}_{gi}")
                xf32[(b, gi)] = xf
                for c in range(c0, c0 + g):
                    xtiles[(b, c)] = (xt, (c - c0) * D)
                c0 += g
        for gi, g in enumerate(LGROUPS):
            c0 = sum(LGROUPS[:gi])
            for b in range(BPC):
                lo = (b * NCH + c0) * D
                nc.sync.dma_start(xf32[(b, gi)][:], xin[:, lo:lo + g * D])

        # -------- constants (gpsimd Q7, between load descriptor jobs) ------
        # Output-row rotation: psum row p holds t'=p-1; row 0 holds t'=126.
        # ramp: psum += sum_k L[k,j]*U4[k,(q,p)] = -1000*max(j - t'(p) - 1, 0)
        lmat = smalls.tile([T, T], BF16)
        nc.gpsimd.memset(lmat[:], 1.0)
        nc.gpsimd.affine_select(
            out=lmat[:], in_=lmat[:], compare_op=ALU.is_ge, fill=0.0,
            base=-1, pattern=[[1, T]], channel_multiplier=-1)
        umat4 = smalls.tile([T, G4, TC], BF16)
        nc.gpsimd.memset(umat4[:], -1000.0)
        nc.gpsimd.affine_select(
            out=umat4[:], in_=umat4[:], compare_op=ALU.is_ge, fill=0.0,
            base=0, pattern=[[0, G4], [-1, TC]], channel_multiplier=1)
        nc.gpsimd.affine_select(
            out=umat4[:], in_=umat4[:], compare_op=ALU.is_ge, fill=0.0,
            base=-1, pattern=[[0, G4], [1, TC]], channel_multiplier=0)
        # block indicator: blk[k, (q, p)] = 1{k == q mod G4}  (bias spread)
        blk = smalls.tile([2 * G4, G4, TC], BF16)
        nc.gpsimd.memset(blk[:], 0.0)
        nc.gpsimd.affine_select(
            out=blk[:], in_=blk[:], compare_op=ALU.not_equal, fill=1.0,
            base=0, pattern=[[-1, G4], [0, TC]], channel_multiplier=1)
        nc.gpsimd.affine_select(
            out=blk[:], in_=blk[:], compare_op=ALU.not_equal, fill=1.0,
            base=-G4, pattern=[[-1, G4], [0, TC]], channel_multiplier=1)

        # -------- gate math (tiny, [SEG, TC]) ------------------------------
        ones_st = smalls.tile([SEG, TC], F32)
        nc.vector.memset(ones_st[:], 1.0)
        ones_row = smalls.tile([2, T], BF16)
        nc.vector.memset(ones_row[:], 1.0)

        om = smalls.tile([SEG, TC], F32)
        av = smalls.tile([SEG, TC], F32)
        bv = smalls.tile([SEG, TC], F32)
        nc.vector.tensor_scalar(om[:], ogt[:], -1.0, 1.0, ALU.mult, ALU.add)
        nc.vector.tensor_scalar(av[:], pgt[:], -1.0, 1.0, ALU.mult, ALU.add)
        nc.vector.tensor_mul(av[:], av[:], om[:])
        nc.vector.tensor_mul(bv[:], pgt[:], om[:])
        nc.vector.tensor_scalar(av[:], av[:], 1e-30, None, ALU.max)
        nc.vector.tensor_scalar(bv[:], bv[:], 1e-30, None, ALU.max)

        la = smalls.tile([SEG, TC], F32)
        nc.scalar.activation(la[:], av[:], ACTF.Ln)
        lb = smalls.tile([SEG, TC], F32)
        nc.scalar.activation(lb[:], bv[:], ACTF.Ln)

        sv = smalls.tile([SEG, TC], F32)
        nc.vector.tensor_tensor_scan(sv[:], ones_st[:], la[:], 0.0,
                                     ALU.mult, ALU.add)
        bias = smalls.tile([SEG, TC], F32)
        nc.vector.tensor_sub(bias[:], lb[:], sv[:])

        # hi/lo bf16 split (fp32 matmuls are dual-pass; two bf16 matmuls
        # reconstruct fp32-accurate sums in the f32 PSUM)
        svh = smalls.tile([SEG, TC], BF16)
        svl = smalls.tile([SEG, TC], BF16)
        nc.vector.tensor_copy(svh[:], sv[:])
        nc.vector.tensor_sub(svl[:], sv[:], svh[:])
        bih = smalls.tile([SEG, TC], BF16)
        bil = smalls.tile([SEG, TC], BF16)
        nc.vector.tensor_copy(bih[:], bias[:])
        nc.vector.tensor_sub(bil[:], bias[:], bih[:])

        # rotate t' by one in SBUF (free-dim copies), then partition-0 row
        # layout via direct SBUF->SBUF DMAs (scalar HWDGE ring)
        svhr = smalls.tile([SEG, TC], BF16)
        svlr = smalls.tile([SEG, TC], BF16)
        for dst, src in ((svhr, svh), (svlr, svl)):
            nc.vector.tensor_copy(dst[:, 0:1], src[:, TC - 1:TC])
            nc.vector.tensor_copy(dst[:, 1:TC], src[:, 0:TC - 1])
        srows2 = smalls.tile([2, SEG * TC], BF16)
        r2 = srows2[:].rearrange("a (p f) -> a p f", f=TC)
        nc.scalar.dma_start(r2[0:1], svhr[:])
        nc.scalar.dma_start(r2[1:2], svlr[:])

        # grouped bias via DRAM bounce; j'=0 (carry row) bias is 0
        bghl = smalls.tile([2 * G4, 2 * NG, T], BF16)
        nc.vector.memset(bghl[:, :, 0:1], 0.0)
        nc.scalar.dma_start(scr_bh[:].rearrange("o (p f) -> (o p) f", f=TC),
                            bih[:])
        nc.scalar.dma_start(scr_bl[:].rearrange("o (p f) -> (o p) f", f=TC),
                            bil[:])
        nc.scalar.dma_start(
            bghl[0:G4, :, 1:T],
            scr_bh[:].rearrange("o (g p f) -> (o p) g f", p=G4, f=TC))
        nc.scalar.dma_start(
            bghl[G4:2 * G4, :, 1:T],
            scr_bl[:].rearrange("o (g p f) -> (o p) g f", p=G4, f=TC))

        # -------- PE warmup: back-to-back dummy matmuls during load window
        nwarm = int(os.environ.get("DSTACK_WARM", "35"))
        for wi in range(nwarm):
            pw = pspool.tile([T, G4 * TC], F32, tag="p2", bufs=2,
                             name=f"warm{wi}")
            nc.tensor.matmul(pw[:, 0:G4 * TC], lmat[:],
                             umat4[:].rearrange("p a b -> p (a b)"),
                             start=True, stop=True)

        # -------- store plan -----------------------------------------------
        sgrp = {}
        for b in range(BPC):
            c0 = 0
            for gi, g in enumerate(SGROUPS):
                yt = ypool.tile([T, g * D], F32, tag=f"y{b}_{gi}")
                for c in range(c0, c0 + g):
                    sgrp[(b, c)] = (yt, (c - c0) * D, c == c0 + g - 1,
                                    (b * NCH + c0) * D, g)
                c0 += g

        # -------- main loop ------------------------------------------------
        cts = {}
        psys = {}
        gstart = {}
        for gi in range(len(LGROUPS)):
            st = sum(LGROUPS[:gi])
            gstart.setdefault(max(0, st - 1), []).append(gi)
        for c in range(NCH):
            for b in range(BPC):
                seg = b * SEGP + c
                q = c // G4
                for gi in gstart.get(c, ()):
                    g = LGROUPS[gi]
                    xf = xf32[(b, gi)]
                    xtt = xtiles[(b, sum(LGROUPS[:gi]))][0]
                    nc.vector.tensor_copy(xtt[:, 0:g * D], xf[:, 0:g * D])
                if c % G4 == 0:
                    gsz = min(G4, NCH - c)
                    w = gsz * TC
                    ps2 = pspool.tile([T, G4 * TC], F32, tag="p2", bufs=2,
                                      name=f"ps2_{b}_{c}")
                    nc.tensor.matmul(ps2[:, 0:w], ones_row[:, 0:T],
                                     srows2[:, seg * TC:(seg + gsz) * TC],
                                     start=True, stop=False)
                    nc.tensor.matmul(
                        ps2[:, 0:w], bghl[:, b * NG + q, :],
                        blk[:].rearrange("p a b -> p (a b)")[:, 0:w],
                        start=False, stop=False, skip_group_check=True)
                    nc.tensor.matmul(
                        ps2[:, 0:w], lmat[:],
                        umat4[:].rearrange("p a b -> p (a b)")[:, 0:w],
                        start=False, stop=True, skip_group_check=True)
                    ct = ctpool.tile([T, G4 * TC], BF16, tag=f"ct{b}",
                                     name=f"ct_{b}_{c}")
                    nc.scalar.activation(ct[:, 0:w], ps2[:, 0:w], ACTF.Exp)
                    cts[(b, q)] = ct

                ct = cts[(b, q)]
                xt, xcol = xtiles[(b, c)]
                if c % PSYC == 0:
                    psy = pspool.tile([T, PSYC * D], F32, tag="psy",
                                      bufs=PSYBUFS, name=f"psy{b}_{c}")
                    psys[b] = psy
                psy = psys[b]
                pcol = (c % PSYC) * D
                nc.tensor.matmul(psy[0:TC, pcol:pcol + D],
                                 ct[:, (c % G4) * TC:(c % G4 + 1) * TC],
                                 xt[:, xcol:xcol + D],
                                 start=True, stop=True)

                # carry for the next chunk: rotated row 0 holds t'=126; its
                # own carry coefficient is exp(S_126) ~ e^-264 = 0, so this
                # never waits on the previous carry.  Write it straight into
                # row 0 (the carry slot) of the next chunk's x tile.
                if c < NCH - 1:
                    xt1, xcol1 = xtiles[(b, c + 1)]
                    if c % 2 == 0:
                        nc.scalar.copy(xt1[0:1, xcol1:xcol1 + D],
                                       psy[0:1, pcol:pcol + D])
                    else:
                        nc.vector.tensor_copy(xt1[0:1, xcol1:xcol1 + D],
                                              psy[0:1, pcol:pcol + D])

                # psum group -> sbuf y (split ScalarE / DVE), once per group
                if c % PSYC == PSYC - 1 or c == NCH - 1:
                    cg0 = c - (c % PSYC)
                    yt, ycol, _, _, _ = sgrp[(b, cg0)]
                    w = (c % PSYC + 1) * D
                    nds = DVE_COLS * w // 512
                    nc.scalar.copy(yt[:, ycol:ycol + w - nds],
                                   psy[:, 0:w - nds])
                    if nds:
                        nc.vector.tensor_copy(yt[:, ycol + w - nds:ycol + w],
                                              psy[:, w - nds:w])
                yt, ycol, last, dcol0, g = sgrp[(b, c)]
                if last:
                    # stored rotated (row p holds t'=p-1); host un-rotates
                    nc.sync.dma_start(yout[:, dcol0:dcol0 + g * D], yt[:])

    nc.compile()
    return nc


_module_cache = {}


def _get_module():
    if "nc" not in _module_cache:
        _module_cache["nc"] = build_module()
    return _module_cache["nc"]


def make_in_maps(x, push_gate, pop_gate):
    x = np.ascontiguousarray(np.asarray(x), dtype=np.float32)
    pgf = np.asarray(push_gate, dtype=np.float32).reshape(B, L)
    ogf = np.asarray(pop_gate, dtype=np.float32).reshape(B, L)
    LP = NCH * TC                      # padded length = 2159
    xp = np.zeros((B, LP + 1, D), dtype=np.float32)
    xp[:, 1:L + 1] = x                 # shift by one: row 0 = carry slot
    gp = np.zeros((B, LP), dtype=np.float32)
    op_ = np.zeros((B, LP), dtype=np.float32)
    gp[:, :L] = pgf
    op_[:, :L] = ogf
    in_maps = []
    for i in range(NCORES):
        sl = slice(i * BPC, (i + 1) * BPC)
        # xin[p, b, c, d] = xp[b, TC*c + p]  (p=0 -> carry slot)
        xi = np.zeros((T, BPC, NCH, D), dtype=np.float32)
        xpc = xp[sl]
        for c in range(NCH):
            xi[:, :, c, :] = xpc[:, TC * c:TC * c + T].transpose(1, 0, 2)
        gg = np.zeros((SEG, TC), dtype=np.float32)
        oo = np.zeros((SEG, TC), dtype=np.float32)
        gg.reshape(BPC, SEGP, TC)[:, :NCH] = gp[sl].reshape(BPC, NCH, TC)
        oo.reshape(BPC, SEGP, TC)[:, :NCH] = op_[sl].reshape(BPC, NCH, TC)
        in_maps.append({
            "xin": np.ascontiguousarray(xi.reshape(T, BPC * NCH * D)),
            "pg": np.ascontiguousarray(gg),
            "og": np.ascontiguousarray(oo),
        })
    return in_maps


def run(x, push_gate, pop_gate, **spmd_kwargs):
    """Run on hardware; returns (output, BassKernelResults)."""
    nc = _get_module()
    in_maps = make_in_maps(x, push_gate, pop_gate)
    res = run_bass_kernel_spmd(nc, in_maps, core_ids=list(range(NCORES)),
                               **spmd_kwargs)
    outs = []
    for i in range(NCORES):
        yo = res.results[i]["yout"].reshape(T, BPC, NCH, D)
        # row p holds t'=p-1 (row 0 holds t'=TC-1); un-rotate and trim
        yr = np.concatenate([yo[1:TC], yo[0:1]], axis=0)
        y = yr.transpose(1, 2, 0, 3).reshape(BPC, NCH * TC, D)[:, :L]
        outs.append(y)
    return np.concatenate(outs, axis=0), res


def kernel(x, push_gate, pop_gate):
    out, _ = run(x, push_gate, pop_gate)
    return out
